# revision 3
# baseline (speedup 1.0000x reference)
"""Trainium2 Bass kernel for nn_CNNQNetwork (dueling CNN Q-network).

Sharding: pure data parallel — batch 4096 split as 512 samples on each of the
8 NeuronCores; all weights replicated.

Per-core layout: activations live in SBUF as [channel(partition), spatial, batch].
Convs are TensorE matmuls (one per kernel tap, accumulated in PSUM).
GroupNorm(1 group) per sample:
  - per-(c,b) partial sums via DVE tensor_reduce, squares via ScalarE
  - cross-channel reduction via tiny ones-matmuls (TensorE)
  - mean subtraction folded back into the conv PSUM as a K=1 matmul
  - gamma*rstd applied as a broadcast matmul + GpSimd multiply while casting
    features to bf16 for the MLP head matmuls
The dueling-head algebra (v + a - mean(a), biases) is folded into the second
linear layer's weights on the host.

Math notes used for exactness (verified against the reference):
  - relu(GroupNorm) with gamma>0, beta=0, conv bias=0 allows deferring the
    per-sample 1/std into the *feature* tensor only; intermediate blocks are
    scale invariant because GroupNorm(conv(r*u)) == GroupNorm(conv(u)).
  - per-channel gamma of h1/v1 is folded into the consuming conv weights.
"""

import numpy as np
import ml_dtypes

BF16 = ml_dtypes.bfloat16
B_TOTAL = 4096
NCORES = 8
BC = B_TOTAL // NCORES  # 512 samples per core
D = 128
EPS = 1e-5

# blocks: (name, src, kind, Hi, Wi, Ho, Wo)   kind 'h' = (1,2) kernel, 'v' = (2,1)
BLOCKS = [
    ("h1", "x2", "h", 4, 4, 4, 3),
    ("v1", "x3", "v", 4, 4, 3, 4),
    ("hh", "h1", "h", 4, 3, 4, 2),
    ("hv", "h1", "v", 4, 3, 3, 3),
    ("vh", "v1", "h", 3, 4, 3, 3),
    ("vv", "v1", "v", 3, 4, 2, 4),
]
S_OF = {n: ho * wo for (n, _, _, _, _, ho, wo) in BLOCKS}
NK = sum(S_OF.values())  # 58 K-slices of 128 for the head matmul

_cache = {}


def _build(loop_n=None):
    """Build the Bass program once. Returns (nc, meta)."""
    import concourse.bass as bass
    import concourse.tile as tile
    import concourse.mybir as mybir
    from concourse import bacc
    from concourse.masks import make_identity
    from contextlib import ExitStack

    dt = mybir.dt
    Alu = mybir.AluOpType
    Act = mybir.ActivationFunctionType

    from contextlib import nullcontext

    nc = bacc.Bacc(
        "TRN2",
        target_bir_lowering=False,
        debug=False,
        enable_asserts=False,
        num_devices=NCORES,
    )

    # ---- DRAM I/O ----
    x2_d = nc.dram_tensor("x2", [32, 16, BC], dt.bfloat16, kind="ExternalInput")
    x3_d = nc.dram_tensor("x3", [32, 16, BC], dt.bfloat16, kind="ExternalInput")
    cw1_d = nc.dram_tensor("cw1", [32, 256], dt.bfloat16, kind="ExternalInput")
    cw_d = nc.dram_tensor("cw", [128, 8 * 128], dt.bfloat16, kind="ExternalInput")
    hw_d = nc.dram_tensor("hw", [4, 128, NK * 128], dt.bfloat16, kind="ExternalInput")
    fw_d = nc.dram_tensor("fw", [128, 16], dt.bfloat16, kind="ExternalInput")
    hb_d = nc.dram_tensor("hb", [128, 4], dt.float32, kind="ExternalInput")
    b2_d = nc.dram_tensor("b2", [4, 1], dt.float32, kind="ExternalInput")
    gam_d = nc.dram_tensor("gam", [1, 6 * 128], dt.bfloat16, kind="ExternalInput")
    out_d = nc.dram_tensor("out", [BC, 4], dt.float32, kind="ExternalOutput")

    with tile.TileContext(nc) as tc, ExitStack() as ctx:
        singles = ctx.enter_context(tc.tile_pool(name="singles", bufs=1))
        rows = ctx.enter_context(tc.tile_pool(name="rows", bufs=4))
        sqp = ctx.enter_context(tc.tile_pool(name="sqp", bufs=6))
        stats = ctx.enter_context(tc.tile_pool(name="stats", bufs=2))
        uleafp = ctx.enter_context(tc.tile_pool(name="uleafp", bufs=2))

        # persistent SBUF tensors
        fw_sb = singles.tile([128, 16], dt.bfloat16, tag="fw", name="fw")
        hb_sb = singles.tile([128, 4], dt.float32, tag="hb", name="hb")
        b2_sb = singles.tile([4, 1], dt.float32, tag="b2", name="b2")
        gam_sb = singles.tile([1, 6 * 128], dt.bfloat16, tag="gam", name="gam")
        ident = singles.tile([128, 128], dt.float32, tag="ident", name="ident")
        ones_bf = singles.tile([128, 1], dt.bfloat16, tag="ones", name="ones")
        eps1 = singles.tile([1, 1], dt.float32, tag="eps1", name="eps1")
        nc.vector.memset(eps1[:], EPS)

        nc.sync.dma_start(fw_sb[:], fw_d[:])
        nc.sync.dma_start(hb_sb[:], hb_d[:])
        nc.sync.dma_start(b2_sb[:], b2_d[:])
        nc.sync.dma_start(gam_sb[:], gam_d[:])
        make_identity(nc, ident[:])
        nc.vector.memset(ones_bf[:], 1.0)

        # feature tiles (bf16, [c, s, b]) and the conv-chain activations
        feat = {}
        for name, _, _, _, _, ho, wo in BLOCKS:
            feat[name] = singles.tile([128, ho * wo, BC], dt.bfloat16, tag=f"f_{name}", name=f"f_{name}")
        u_keep = {
            "h1": singles.tile([128, 12, BC], dt.bfloat16, tag="u_h1", name="u_h1"),
            "v1": singles.tile([128, 12, BC], dt.bfloat16, tag="u_v1", name="u_v1"),
        }

        with (tc.For_i(0, loop_n, 1) if loop_n else nullcontext()):
            with (
                tc.tile_pool(name="convp", bufs=1) as convp,
                tc.tile_pool(name="zp", bufs=5, space="PSUM") as zp,
                tc.tile_pool(name="sp", bufs=2, space="PSUM") as sp,
                tc.tile_pool(name="gp", bufs=1, space="PSUM") as gp,
            ):
                x2_sb = convp.tile([32, 16, BC], dt.bfloat16, tag="x2", name="x2")
                x3_sb = convp.tile([32, 16, BC], dt.bfloat16, tag="x3", name="x3")
                cw1_sb = convp.tile([32, 256], dt.bfloat16, tag="cw1", name="cw1")
                cw_sb = convp.tile([128, 8 * 128], dt.bfloat16, tag="cw", name="cw")
                nc.sync.dma_start(x2_sb[:], x2_d[:])
                nc.sync.dma_start(x3_sb[:], x3_d[:])
                nc.sync.dma_start(cw1_sb[:], cw1_d[:])
                nc.sync.dma_start(cw_sb[:], cw_d[:])
                for bi, (name, src, kind, Hi, Wi, Ho, Wo) in enumerate(BLOCKS):
                    S = Ho * Wo
                    CS = 128 * S
                    first = src in ("x2", "x3")

                    if first:
                        sview = (x2_sb if src == "x2" else x3_sb)[:].rearrange(
                            "c (i j) b -> c i j b", i=Hi
                        )
                    else:
                        sview = u_keep[src][:].rearrange("c (i j) b -> c i j b", i=Hi)

                    def rhs_win(tap, b0, bn):
                        if kind == "h":
                            return sview[:, :, tap : tap + Wo, b0 : b0 + bn]
                        else:
                            return sview[:, tap : tap + Ho, :, b0 : b0 + bn]

                    u_dst = u_keep[name] if name in u_keep else uleafp.tile(
                        [128, S, BC], dt.bfloat16, tag="uleaf", name="uleaf"
                    )

                    zs = stats.tile([128, BC], dt.bfloat16, tag="zs", name="zs")
                    sqs = stats.tile([128, BC], dt.bfloat16, tag="sqs", name="sqs")
                    rrow = stats.tile([1, BC], dt.bfloat16, tag="rrow", name="rrow")

                    negl = rows.tile([1, 128], dt.bfloat16, tag="negl", name="negl")
                    nc.vector.memset(negl[:], -1.0 / CS)

                    for q in range(4):
                        q0 = q * 128
                        psS = sp.tile([1, 2, 128], dt.float32, tag="psS", name="psS")
                        zchunks = []
                        for chi in range(4):
                            b0 = q0 + chi * 32
                            zc = zp.tile([128, S, 32], dt.float32, tag="z", name="z")
                            zchunks.append((b0, zc))
                            if first:
                                lhsT = cw1_sb[:, bi * 128 : bi * 128 + 128]
                                nc.tensor.matmul(
                                    zc[:], lhsT, rhs_win(0, b0, 32), start=True, stop=True
                                )
                            else:
                                t0 = (bi - 2) * 2
                                for t in range(2):
                                    lhsT = cw_sb[:, (t0 + t) * 128 : (t0 + t + 1) * 128]
                                    nc.tensor.matmul(
                                        zc[:],
                                        lhsT,
                                        rhs_win(t, b0, 32),
                                        start=(t == 0),
                                        stop=(t == 1),
                                    )
                        with nc.allow_low_precision("bf16 groupnorm partial sums"):
                            for b0, zc in zchunks:
                                zt = zc[:].rearrange("c s b -> c b s")
                                nc.vector.tensor_reduce(
                                    zs[:, b0 : b0 + 32],
                                    zt,
                                    axis=mybir.AxisListType.X,
                                    op=Alu.add,
                                )
                                sq = sqp.tile([128, S, 32], dt.bfloat16, tag="sq", name="sq")
                                nc.scalar.square(sq[:], zc[:])
                                nc.vector.tensor_reduce(
                                    sqs[:, b0 : b0 + 32],
                                    sq[:].rearrange("c s b -> c b s"),
                                    axis=mybir.AxisListType.X,
                                    op=Alu.add,
                                )
                        # cross-channel sums: psS[0] = sum(zs), psS[1] = sum(sqs)
                        nc.tensor.matmul(
                            psS[:, 0, :], ones_bf[:], zs[:, q0 : q0 + 128],
                            start=True, stop=True,
                        )
                        nc.tensor.matmul(
                            psS[:, 1, :], ones_bf[:], sqs[:, q0 : q0 + 128],
                            start=True, stop=True,
                        )
                        # mu^2 = (sA/CS)^2 ; ve = sB/CS - mu^2 ; sd = sqrt(ve+eps); r = 1/sd
                        srow = rows.tile([1, 2, 128], dt.float32, tag="srow", name="srow")
                        nc.vector.tensor_copy(srow[:], psS[:])
                        mu2 = rows.tile([1, 128], dt.float32, tag="mu2", name="mu2")
                        nc.vector.scalar_tensor_tensor(
                            mu2[:], srow[:, 0, :], 1.0 / (CS * CS), srow[:, 0, :],
                            op0=Alu.mult, op1=Alu.mult,
                        )
                        ve = rows.tile([1, 128], dt.float32, tag="ve", name="ve")
                        nc.vector.scalar_tensor_tensor(
                            ve[:], srow[:, 1, :], 1.0 / CS, mu2[:],
                            op0=Alu.mult, op1=Alu.subtract,
                        )
                        sd = rows.tile([1, 128], dt.float32, tag="sd", name="sd")
                        nc.scalar.activation(
                            sd[:], ve[:], func=Act.Sqrt, bias=eps1[:], scale=1.0
                        )
                        rt = rows.tile([1, 128], dt.float32, tag="rt", name="rt")
                        nc.vector.reciprocal(rt[:], sd[:])
                        nc.vector.tensor_copy(rrow[:, q0 : q0 + 128], rt[:])
                        # bf16 row of sum(z) for the mean-subtract matmul
                        zrow = rows.tile([1, 128], dt.bfloat16, tag="zrow", name="zrow")
                        nc.vector.tensor_copy(zrow[:], srow[:, 0, :])
                        zq = rows.tile([1, S, 128], dt.bfloat16, tag="zq", name="zq")
                        nc.sync.dma_start(
                            zq[:], zrow[:, None, :].to_broadcast((1, S, 128))
                        )
                        # z -= mean  (K=1 matmul, lhsT = -1/CS)
                        for chi, (b0, zc) in enumerate(zchunks):
                            nc.tensor.matmul(
                                zc[:],
                                negl[:],
                                zq[:, :, chi * 32 : (chi + 1) * 32],
                                start=False,
                                stop=True,
                                skip_group_check=True,
                            )
                        # u = relu(z - mean)
                        for b0, zc in zchunks:
                            nc.scalar.activation(
                                u_dst[:, :, b0 : b0 + 32], zc[:], func=Act.Relu
                            )
                    # G[c,b] = gamma_c * r_b  (broadcast matmul), then feat = u * G
                    psG = gp.tile([128, BC], dt.float32, tag="psG", name="psG")
                    nc.tensor.matmul(
                        psG[:], gam_sb[:, bi * 128 : (bi + 1) * 128], rrow[:],
                        start=True, stop=True,
                    )
                    gsb = stats.tile([128, BC], dt.bfloat16, tag="gsb", name="gsb")
                    nc.scalar.copy(gsb[:], psG[:])
                    nc.gpsimd.tensor_tensor(
                        feat[name][:],
                        u_dst[:],
                        gsb[:, None, :].to_broadcast((128, S, BC)),
                        op=Alu.mult,
                    )

            # ---- heads ----
            with (
                tc.tile_pool(name="hwp", bufs=2) as hwp,
                tc.tile_pool(name="hidp", bufs=1) as hidp,
                tc.tile_pool(name="hp", bufs=2, space="PSUM") as hp,
                tc.tile_pool(name="fp", bufs=1, space="PSUM") as fp,
                tc.tile_pool(name="tp", bufs=2, space="PSUM") as tp,
            ):
                hids = []
                for mt in range(4):
                    hws = hwp.tile([128, NK * 128], dt.bfloat16, tag="hws", name="hws")
                    nc.sync.dma_start(hws[:], hw_d[mt])
                    psH = hp.tile([128, BC], dt.float32, tag="psH", name="psH")
                    k = 0
                    for name, _, _, _, _, ho, wo in BLOCKS:
                        for s in range(ho * wo):
                            nc.tensor.matmul(
                                psH[:],
                                hws[:, k * 128 : (k + 1) * 128],
                                feat[name][:, s, :],
                                start=(k == 0),
                                stop=(k == NK - 1),
                            )
                            k += 1
                    hid = hidp.tile([128, BC], dt.bfloat16, tag=f"hid{mt}", name=f"hid{mt}")
                    nc.scalar.activation(
                        hid[:], psH[:], func=Act.Relu, bias=hb_sb[:, mt : mt + 1], scale=1.0
                    )
                    hids.append(hid)
                psF = fp.tile([4, BC], dt.float32, tag="psF", name="psF")
                for mt in range(4):
                    nc.tensor.matmul(
                        psF[:],
                        fw_sb[:, mt * 4 : (mt + 1) * 4],
                        hids[mt][:],
                        start=(mt == 0),
                        stop=(mt == 3),
                    )
                finf = rows.tile([4, BC], dt.float32, tag="finf", name="finf")
                nc.scalar.activation(
                    finf[:], psF[:], func=Act.Identity, bias=b2_sb[:, 0:1], scale=1.0
                )
                osb = rows.tile([128, 4, 4], dt.float32, tag="osb", name="osb")
                for qq in range(4):
                    psT = tp.tile([128, 4], dt.float32, tag="psT", name="psT")
                    nc.tensor.transpose(
                        psT[:], finf[:, qq * 128 : (qq + 1) * 128], ident[0:4, 0:4]
                    )
                    nc.scalar.copy(osb[:, qq, :], psT[:])
                nc.sync.dma_start(out_d[:].rearrange("(q p) j -> p q j", p=128), osb[:])

    nc.compile()
    return nc


def _prep_weights(inp):
    """Host-side weight preprocessing shared by all cores."""
    f32 = np.float32
    for k in ("b_h1", "b_v1", "b_hh", "b_hv", "b_vh", "b_vv"):
        assert np.allclose(inp[k], 0.0), f"conv bias {k} must be zero"
    for k in ("gb_h1", "gb_v1", "gb_hh", "gb_hv", "gb_vh", "gb_vv"):
        assert np.allclose(inp[k], 0.0), f"groupnorm beta {k} must be zero"
    gammas = {n: np.asarray(inp[f"gw_{n}"], f32) for n in S_OF}
    for n, g in gammas.items():
        assert np.all(g > 0), f"gamma {n} must be positive"

    # first-level conv lhsT (taps stacked into K=32)
    w_h1 = np.asarray(inp["w_h1"], f32)
    w_v1 = np.asarray(inp["w_v1"], f32)
    cw1 = np.zeros((32, 256), f32)
    cw1[0:16, 0:128] = w_h1[:, :, 0, 0].T
    cw1[16:32, 0:128] = w_h1[:, :, 0, 1].T
    cw1[0:16, 128:256] = w_v1[:, :, 0, 0].T
    cw1[16:32, 128:256] = w_v1[:, :, 1, 0].T

    # second-level conv lhsT with parent's gamma folded in
    cw = np.zeros((128, 8 * 128), f32)
    second = [
        ("hh", "w_hh", "h1", "h"),
        ("hv", "w_hv", "h1", "v"),
        ("vh", "w_vh", "v1", "h"),
        ("vv", "w_vv", "v1", "v"),
    ]
    for idx, (name, wk, parent, kind) in enumerate(second):
        w = np.asarray(inp[wk], f32)
        g = gammas[parent]
        for t in range(2):
            tap = w[:, :, 0, t] if kind == "h" else w[:, :, t, 0]
            cw[:, (2 * idx + t) * 128 : (2 * idx + t + 1) * 128] = (tap * g[None, :]).T

    # head weights: W1c = [vw1; aw1] (512, 7424), re-tiled per (mtile, block, s)
    W1c = np.concatenate(
        [np.asarray(inp["vw1"], f32), np.asarray(inp["aw1"], f32)], axis=0
    )
    cols = []
    off = 0
    for name, _, _, _, _, ho, wo in BLOCKS:
        S = ho * wo
        Wb = W1c[:, off : off + 128 * S].reshape(512, 128, S)
        off += 128 * S
        for s in range(S):
            cols.append(Wb[:, :, s])
    K = np.stack(cols, 0)  # (58, 512, 128c)
    hw = np.empty((4, 128, NK * 128), f32)
    for mt in range(4):
        hw[mt] = K[:, mt * 128 : (mt + 1) * 128, :].transpose(2, 0, 1).reshape(128, -1)

    # final layer with dueling algebra folded in
    vw2 = np.asarray(inp["vw2"], f32)  # (1, 256)
    aw2 = np.asarray(inp["aw2"], f32)  # (4, 256)
    W2c = np.zeros((4, 512), f32)
    W2c[:, 0:256] = vw2[0][None, :]
    W2c[:, 256:512] = aw2 - aw2.mean(axis=0, keepdims=True)
    W2cT = W2c.T  # (512, 4)
    fw = np.zeros((128, 16), f32)
    for kt in range(4):
        fw[:, kt * 4 : (kt + 1) * 4] = W2cT[kt * 128 : (kt + 1) * 128, :]
    b2 = (
        np.asarray(inp["vb2"], f32)[0]
        + np.asarray(inp["ab2"], f32)
        - np.asarray(inp["ab2"], f32).mean()
    ).reshape(4, 1)

    hb = np.concatenate(
        [np.asarray(inp["vb1"], f32), np.asarray(inp["ab1"], f32)]
    ).reshape(4, 128).T.copy()  # [128, 4], column mt

    gam = np.zeros((1, 6 * 128), f32)
    for bi, (name, _, _, _, _, _, _) in enumerate(BLOCKS):
        gam[0, bi * 128 : (bi + 1) * 128] = gammas[name]

    return {
        "cw1": cw1.astype(BF16),
        "cw": cw.astype(BF16),
        "hw": hw.astype(BF16),
        "fw": fw.astype(BF16),
        "hb": hb.astype(np.float32),
        "b2": b2.astype(np.float32),
        "gam": gam.astype(BF16),
    }


def _prep_x(xs):
    """Per-core input prep: build the tap-stacked, [c,s,b] bf16 arrays."""
    f32 = np.float32
    n = xs.shape[0]
    x2 = np.zeros((n, 32, 4, 4), f32)
    x2[:, 0:16] = xs
    x2[:, 16:32, :, 0:3] = xs[:, :, :, 1:4]
    x3 = np.zeros((n, 32, 4, 4), f32)
    x3[:, 0:16] = xs
    x3[:, 16:32, 0:3, :] = xs[:, :, 1:4, :]
    x2 = x2.transpose(1, 2, 3, 0).reshape(32, 16, n)
    x3 = x3.transpose(1, 2, 3, 0).reshape(32, 16, n)
    return x2.astype(BF16), x3.astype(BF16)


def _prep_x_map(xs, w):
    """Per-core input map: shared weights + this core's prepped x views."""
    x2, x3 = _prep_x(xs)
    m = dict(w)
    m["x2"] = x2
    m["x3"] = x3
    return m


def _get_nc():
    if "nc" not in _cache:
        _cache["nc"] = _build()
    return _cache["nc"]


def kernel(**inputs) -> np.ndarray:
    from concourse.bass_utils import run_bass_kernel_spmd

    nc = _get_nc()
    x = np.asarray(inputs["x"], np.float32)
    w = _prep_weights(inputs)

    in_maps = []
    for c in range(NCORES):
        xs = x[c * BC : (c + 1) * BC]
        in_maps.append(_prep_x_map(xs, w))

    res = run_bass_kernel_spmd(nc, in_maps, core_ids=list(range(NCORES)))
    out = np.concatenate([r["out"] for r in res.results], axis=0)
    return out.astype(np.float32)



# revision 12
# speedup vs baseline: 1.5354x; 1.5354x over previous
"""Trainium2 Bass kernel for nn_CNNQNetwork (dueling CNN Q-network).

Sharding: pure data parallel — batch 4096 split as 512 samples on each of the
8 NeuronCores; all weights replicated.

v2 design (vs baseline): activations stay [channel, spatial, batch] in SBUF,
but the GroupNorm pipeline is restructured so PSUM frees immediately and the
DVE/Scalar engines run few, large instructions:

  - Mean subtraction uses a parent-derived correction: sum_{c,s} conv(u) is a
    linear functional of the parent, computed as colsum(W)^T @ window_sum(u)
    (tiny matmuls) with window sums via a handful of full-width DVE adds.
    The correction is accumulated into the conv PSUM as a K=1 matmul BEFORE
    any stats are read, so the conv->relu chain never waits on statistics.
  - z' (centered) is evacuated PSUM->SBUF bf16 in one ScalarE instruction per
    q-chunk; variance = sum(z'^2) via DVE square + ones-matmul on TensorE.
  - rstd = exp(-0.5*ln(var+eps)) on ScalarE (both funcs in one table set),
    avoiding the banned Rsqrt and the slow DVE reciprocal.
  - feat = max(z',0) * (gamma_c * rstd_b) in ONE DVE scalar_tensor_tensor.
  - Children consume feat directly (GroupNorm is exactly invariant to the
    per-sample rstd scale; gamma is part of the reference activation).
  - Head: 58 K-slices x 4 m-tiles of N=512 matmuls, k-outer so the head
    weight stream is read exactly once; dueling algebra folded into layer 2.
"""

import numpy as np
import ml_dtypes

BF16 = ml_dtypes.bfloat16
B_TOTAL = 4096
NCORES = 8
BC = B_TOTAL // NCORES  # 512 samples per core
D = 128
EPS = 1e-5

# blocks: (name, src, kind, Hi, Wi, Ho, Wo)   kind 'h' = (1,2) kernel, 'v' = (2,1)
BLOCKS = [
    ("h1", "x2", "h", 4, 4, 4, 3),
    ("v1", "x3", "v", 4, 4, 3, 4),
    ("hh", "h1", "h", 4, 3, 4, 2),
    ("hv", "h1", "v", 4, 3, 3, 3),
    ("vh", "v1", "h", 3, 4, 3, 3),
    ("vv", "v1", "v", 3, 4, 2, 4),
]
S_OF = {n: ho * wo for (n, _, _, _, _, ho, wo) in BLOCKS}
NK = sum(S_OF.values())  # 58 K-slices of 128 for the head matmul
SMAX = 12
QN = 4  # four chunks of 128 samples

_cache = {}


def _conv_pieces(kind, Ho, Wo):
    """Bank-safe conv matmul pieces: per output row, split the s-range at
    PSUM bank boundaries (multiples of 4 fp32*128-lane slots = 2KB).
    Returns list of (i, j0, j1) with out slots s in [i*Wo+j0, i*Wo+j1)."""
    pieces = []
    for i in range(Ho):
        j0 = 0
        while j0 < Wo:
            s0 = i * Wo + j0
            # next bank boundary in s-space
            j1 = min(Wo, j0 + (4 - s0 % 4) if s0 % 4 else j0 + 4)
            pieces.append((i, j0, j1))
            j0 = j1
    return pieces


def _k1_pieces(S):
    """Bank-aligned s-ranges covering [0, S) for the K=1 mean-subtract."""
    return [(s0, min(s0 + 4, S)) for s0 in range(0, S, 4)]


def _build(loop_n=None):
    import concourse.bass as bass
    import concourse.tile as tile
    import concourse.mybir as mybir
    from concourse import bacc
    from concourse.masks import make_identity
    from contextlib import ExitStack, nullcontext

    dt = mybir.dt
    Alu = mybir.AluOpType
    Act = mybir.ActivationFunctionType

    nc = bacc.Bacc(
        "TRN2",
        target_bir_lowering=False,
        debug=False,
        enable_asserts=False,
        num_devices=NCORES,
    )

    # ---- DRAM I/O ----
    x2_d = nc.dram_tensor("x2", [32, 16, BC], dt.bfloat16, kind="ExternalInput")
    x3_d = nc.dram_tensor("x3", [32, 16, BC], dt.bfloat16, kind="ExternalInput")
    pwx_d = nc.dram_tensor("pwx", [32, 2, BC], dt.bfloat16, kind="ExternalInput")
    cw1_d = nc.dram_tensor("cw1", [32, 256], dt.bfloat16, kind="ExternalInput")
    cw_d = nc.dram_tensor("cw", [128, 8 * 128], dt.bfloat16, kind="ExternalInput")
    ncol1_d = nc.dram_tensor("ncol1", [32, 2], dt.bfloat16, kind="ExternalInput")
    ncol2_d = nc.dram_tensor("ncol2", [128, 8], dt.bfloat16, kind="ExternalInput")
    gam6_d = nc.dram_tensor("gam6", [1, 6 * 128], dt.float32, kind="ExternalInput")
    hw_d = nc.dram_tensor("hw", [NK, 128, 512], dt.bfloat16, kind="ExternalInput")
    fw_d = nc.dram_tensor("fw", [128, 16], dt.bfloat16, kind="ExternalInput")
    hb_d = nc.dram_tensor("hb", [128, 4], dt.float32, kind="ExternalInput")
    b2_d = nc.dram_tensor("b2", [4, 1], dt.float32, kind="ExternalInput")
    out_d = nc.dram_tensor("out", [BC, 4], dt.float32, kind="ExternalOutput")

    with tile.TileContext(nc) as tc, ExitStack() as ctx:
        singles = ctx.enter_context(tc.tile_pool(name="singles", bufs=1))

        # persistent SBUF tensors
        x2_sb = singles.tile([32, 16, BC], dt.bfloat16, tag="x2", name="x2")
        x3_sb = singles.tile([32, 16, BC], dt.bfloat16, tag="x3", name="x3")
        pwx_sb = singles.tile([32, 2, BC], dt.bfloat16, tag="pwx", name="pwx")
        cw1_sb = singles.tile([32, 256], dt.bfloat16, tag="cw1", name="cw1")
        cw_sb = singles.tile([128, 8 * 128], dt.bfloat16, tag="cw", name="cw")
        ncol1_sb = singles.tile([32, 2], dt.bfloat16, tag="ncol1", name="ncol1")
        ncol2_sb = singles.tile([128, 8], dt.bfloat16, tag="ncol2", name="ncol2")
        gam6_sb = singles.tile([1, 6 * 128], dt.float32, tag="gam6", name="gam6")
        fw_sb = singles.tile([128, 16], dt.bfloat16, tag="fw", name="fw")
        hb_sb = singles.tile([128, 4], dt.float32, tag="hb", name="hb")
        b2_sb = singles.tile([4, 1], dt.float32, tag="b2", name="b2")
        ident = singles.tile([128, 128], dt.float32, tag="ident", name="ident")
        ones_c = singles.tile([128, 1], dt.bfloat16, tag="ones_c", name="ones_c")
        ones_r = singles.tile([1, 128], dt.bfloat16, tag="ones_r", name="ones_r")
        eps1 = singles.tile([1, 1], dt.float32, tag="eps1", name="eps1")
        rstd_sb = singles.tile([1, 6 * BC], dt.float32, tag="rstd", name="rstd")

        nc.sync.dma_start(x2_sb[:], x2_d[:])
        nc.sync.dma_start(x3_sb[:], x3_d[:])
        nc.sync.dma_start(pwx_sb[:], pwx_d[:])
        nc.sync.dma_start(cw1_sb[:], cw1_d[:])
        nc.sync.dma_start(cw_sb[:], cw_d[:])
        nc.sync.dma_start(ncol1_sb[:], ncol1_d[:])
        nc.sync.dma_start(ncol2_sb[:], ncol2_d[:])
        nc.sync.dma_start(gam6_sb[:], gam6_d[:])
        nc.sync.dma_start(fw_sb[:], fw_d[:])
        nc.sync.dma_start(hb_sb[:], hb_d[:])
        nc.sync.dma_start(b2_sb[:], b2_d[:])
        make_identity(nc, ident[:])
        nc.vector.memset(ones_c[:], 1.0)
        nc.vector.memset(ones_r[:], 1.0)
        nc.vector.memset(eps1[:], EPS)

        feat = {}
        for name, _, _, _, _, ho, wo in BLOCKS:
            feat[name] = singles.tile(
                [128, ho * wo, BC], dt.bfloat16, tag=f"f_{name}", name=f"f_{name}"
            )

        with (tc.For_i(0, loop_n, 1) if loop_n else nullcontext()):
            with (
                tc.tile_pool(name="pw", bufs=2) as pwp,
                tc.tile_pool(name="zc", bufs=2) as zcp,
                tc.tile_pool(name="sq", bufs=2) as sqp,
                tc.tile_pool(name="crow", bufs=2) as crowp,
                tc.tile_pool(name="zq", bufs=2) as zqp,
                tc.tile_pool(name="gsb", bufs=2) as gsbp,
                tc.tile_pool(name="zring", bufs=2, space="PSUM") as zring,
                tc.tile_pool(name="gps", bufs=2, space="PSUM") as gps,
            ):
                for bi, (name, src, kind, Hi, Wi, Ho, Wo) in enumerate(BLOCKS):
                    S = Ho * Wo
                    CS = 128 * S
                    first = src in ("x2", "x3")
                    parent = (
                        (x2_sb if src == "x2" else x3_sb) if first else feat[src]
                    )
                    sview = parent[:].rearrange("c (i j) b -> c i j b", i=Hi)

                    # --- correction row: crow[b] = -(1/CS) sum_{c,s} z ---
                    gtile = gps.tile([128, BC], dt.float32, tag="G", name=f"psC_{name}")
                    psC = gtile[0:1, :]
                    if first:
                        pw = pwx_sb[:, (0 if src == "x2" else 1), :]
                        nc.tensor.matmul(
                            psC, ncol1_sb[:, bi : bi + 1], pw, start=True, stop=True
                        )
                    else:
                        # window sums of the parent via full-width DVE adds
                        # col sums C_j (for kind h) or row sums R_i (kind v)
                        if kind == "h":
                            nsum, stride, count = Wi, Wi, Hi  # C_j = sum_i p[i,j]
                        else:
                            nsum, stride, count = Hi, 1, Wi  # R_i = sum_j p[i,j]
                        lines = pwp.tile(
                            [128, 4, BC], dt.bfloat16, tag="lines", name=f"ln_{name}"
                        )
                        for j in range(nsum):
                            base = j * (1 if kind == "h" else Wi)
                            nc.vector.tensor_tensor(
                                lines[:, j, :],
                                parent[:, base, :],
                                parent[:, base + stride, :],
                                op=Alu.add,
                            )
                            for i in range(2, count):
                                nc.vector.tensor_tensor(
                                    lines[:, j, :],
                                    lines[:, j, :],
                                    parent[:, base + i * stride, :],
                                    op=Alu.add,
                                )
                        # window sum for tap t = sum of Wo (or Ho) adjacent lines
                        nwin = Wo if kind == "h" else Ho
                        pwin = pwp.tile(
                            [128, 2, BC], dt.bfloat16, tag="pwin", name=f"pw_{name}"
                        )
                        for t in range(2):
                            nc.vector.tensor_tensor(
                                pwin[:, t, :],
                                lines[:, t, :],
                                lines[:, t + 1, :],
                                op=Alu.add,
                            )
                            for u in range(2, nwin):
                                nc.vector.tensor_tensor(
                                    pwin[:, t, :],
                                    pwin[:, t, :],
                                    lines[:, t + u, :],
                                    op=Alu.add,
                                )
                        for t in range(2):
                            nc.tensor.matmul(
                                psC,
                                ncol2_sb[:, 2 * bi - 4 + t : 2 * bi - 3 + t],
                                pwin[:, t, :],
                                start=(t == 0),
                                stop=(t == 1),
                            )
                    crow = crowp.tile([1, BC], dt.bfloat16, tag="crow", name=f"cr_{name}")
                    nc.scalar.copy(crow[:], psC)
                    zq = zqp.tile([1, SMAX, BC], dt.bfloat16, tag="zq", name=f"zq_{name}")
                    nc.sync.dma_start(
                        zq[:, 0:S, :], crow[:, None, :].to_broadcast((1, S, BC))
                    )

                    # --- conv + mean-sub + evacuate, per q-chunk of 128 ---
                    zc = zcp.tile([128, SMAX, BC], dt.bfloat16, tag="zc", name=f"zc_{name}")
                    pieces = _conv_pieces(kind, Ho, Wo)
                    for q in range(QN):
                        q0 = q * 128
                        Z = zring.tile([128, SMAX, 128], dt.float32, tag="Z", name=f"Z_{name}{q}")
                        for t in range(2 if not first else 1):
                            for (i, j0, j1) in pieces:
                                s0, s1 = i * Wo + j0, i * Wo + j1
                                if first:
                                    lhsT = cw1_sb[:, bi * 128 : bi * 128 + 128]
                                    rhs = sview[:, i, j0:j1, q0 : q0 + 128]
                                else:
                                    t0 = (bi - 2) * 2
                                    lhsT = cw_sb[:, (t0 + t) * 128 : (t0 + t + 1) * 128]
                                    rhs = (
                                        sview[:, i, j0 + t : j1 + t, q0 : q0 + 128]
                                        if kind == "h"
                                        else sview[:, i + t, j0 : j1, q0 : q0 + 128]
                                    )
                                # start=True clears the WHOLE bank's has_written
                                # bits, so only the first piece touching each
                                # 4-slot bank may open it.
                                nc.tensor.matmul(
                                    Z[:, s0:s1, :],
                                    lhsT,
                                    rhs,
                                    start=(t == 0 and s0 % 4 == 0),
                                    stop=False,
                                    skip_group_check=True,
                                )
                        # K=1 mean subtract per bank, then single-instr evac
                        for (s0, s1) in _k1_pieces(S):
                            nc.tensor.matmul(
                                Z[:, s0:s1, :],
                                ones_r[:],
                                zq[:, s0:s1, q0 : q0 + 128],
                                start=False,
                                stop=True,
                                skip_group_check=True,
                            )
                        nc.scalar.copy(zc[:, 0:S, q0 : q0 + 128], Z[:, 0:S, :])

                    # --- stats: var = sum(z'^2)/CS ; rstd = exp(-.5 ln(var+eps))
                    sq = sqp.tile([128, SMAX, BC], dt.bfloat16, tag="sq", name=f"sq_{name}")
                    nc.vector.tensor_tensor(
                        sq[:, 0:S, :], zc[:, 0:S, :], zc[:, 0:S, :], op=Alu.mult
                    )
                    gtile2 = gps.tile([128, BC], dt.float32, tag="G", name=f"psVG_{name}")
                    psV = gtile2[0:1, :]
                    for s in range(S):
                        nc.tensor.matmul(
                            psV, ones_c[:], sq[:, s, :], start=(s == 0), stop=(s == S - 1)
                        )
                    lnr = crowp.tile([1, BC], dt.float32, tag="lnr", name=f"ln_{name}")
                    nc.scalar.activation(
                        lnr[:], psV, func=Act.Ln, bias=eps1[:], scale=1.0 / CS
                    )
                    nc.scalar.activation(
                        rstd_sb[:, bi * BC : (bi + 1) * BC], lnr[:],
                        func=Act.Exp, scale=-0.5,
                    )
                    # G[c,b] = gamma_c * rstd_b  (outer-product matmul, fp32)
                    nc.tensor.matmul(
                        gtile2[:],
                        gam6_sb[:, bi * 128 : (bi + 1) * 128],
                        rstd_sb[:, bi * BC : (bi + 1) * BC],
                        start=True,
                        stop=True,
                        skip_group_check=True,
                    )
                    gsb = gsbp.tile([128, BC], dt.bfloat16, tag="gsb", name=f"g_{name}")
                    nc.scalar.copy(gsb[:], gtile2[:])
                    # feat = max(z',0) * G   (one DVE pass)
                    nc.vector.scalar_tensor_tensor(
                        feat[name][:],
                        zc[:, 0:S, :],
                        0.0,
                        gsb[:, None, :].to_broadcast((128, S, BC)),
                        op0=Alu.max,
                        op1=Alu.mult,
                    )

            # ---- heads ----
            with (
                tc.tile_pool(name="hwp", bufs=8) as hwp,
                tc.tile_pool(name="hidp", bufs=1) as hidp,
                tc.tile_pool(name="hs", bufs=1) as hsp,
                tc.tile_pool(name="hp", bufs=1, space="PSUM") as hp,
                tc.tile_pool(name="fp", bufs=1, space="PSUM") as fp,
                tc.tile_pool(name="tp", bufs=2, space="PSUM") as tp,
            ):
                psH = [
                    hp.tile([128, BC], dt.float32, tag=f"psH{mt}", name=f"psH{mt}")
                    for mt in range(4)
                ]
                k = 0
                for name, _, _, _, _, ho, wo in BLOCKS:
                    for s in range(ho * wo):
                        hwt = hwp.tile([128, 512], dt.bfloat16, tag="hwt", name=f"hw{k}")
                        nc.sync.dma_start(hwt[:], hw_d[k])
                        for mt in range(4):
                            nc.tensor.matmul(
                                psH[mt][:],
                                hwt[:, mt * 128 : (mt + 1) * 128],
                                feat[name][:, s, :],
                                start=(k == 0),
                                stop=(k == NK - 1),
                            )
                        k += 1
                hids = []
                for mt in range(4):
                    hid = hidp.tile([128, BC], dt.bfloat16, tag=f"hid{mt}", name=f"hid{mt}")
                    nc.scalar.activation(
                        hid[:], psH[mt][:], func=Act.Relu,
                        bias=hb_sb[:, mt : mt + 1], scale=1.0,
                    )
                    hids.append(hid)
                psF = fp.tile([4, BC], dt.float32, tag="psF", name="psF")
                for mt in range(4):
                    nc.tensor.matmul(
                        psF[:],
                        fw_sb[:, mt * 4 : (mt + 1) * 4],
                        hids[mt][:],
                        start=(mt == 0),
                        stop=(mt == 3),
                    )
                finf = hsp.tile([4, BC], dt.float32, tag="finf", name="finf")
                nc.scalar.activation(
                    finf[:], psF[:], func=Act.Identity, bias=b2_sb[:, 0:1], scale=1.0
                )
                osb = hsp.tile([128, 4, 4], dt.float32, tag="osb", name="osb")
                for qq in range(4):
                    psT = tp.tile([128, 4], dt.float32, tag="psT", name="psT")
                    nc.tensor.transpose(
                        psT[:], finf[:, qq * 128 : (qq + 1) * 128], ident[0:4, 0:4]
                    )
                    nc.scalar.copy(osb[:, qq, :], psT[:])
                nc.sync.dma_start(out_d[:].rearrange("(q p) j -> p q j", p=128), osb[:])

    nc.compile()
    return nc


def _prep_weights(inp):
    """Host-side weight preprocessing shared by all cores."""
    f32 = np.float32
    for k in ("b_h1", "b_v1", "b_hh", "b_hv", "b_vh", "b_vv"):
        assert np.allclose(inp[k], 0.0), f"conv bias {k} must be zero"
    for k in ("gb_h1", "gb_v1", "gb_hh", "gb_hv", "gb_vh", "gb_vv"):
        assert np.allclose(inp[k], 0.0), f"groupnorm beta {k} must be zero"
    gammas = {n: np.asarray(inp[f"gw_{n}"], f32) for n in S_OF}

    # first-level conv lhsT (taps stacked into K=32)
    w_h1 = np.asarray(inp["w_h1"], f32)
    w_v1 = np.asarray(inp["w_v1"], f32)
    cw1 = np.zeros((32, 256), f32)
    cw1[0:16, 0:128] = w_h1[:, :, 0, 0].T
    cw1[16:32, 0:128] = w_h1[:, :, 0, 1].T
    cw1[0:16, 128:256] = w_v1[:, :, 0, 0].T
    cw1[16:32, 128:256] = w_v1[:, :, 1, 0].T

    # second-level conv lhsT, RAW weights (children consume feat directly)
    cw = np.zeros((128, 8 * 128), f32)
    second = [("hh", "w_hh", "h"), ("hv", "w_hv", "v"),
              ("vh", "w_vh", "h"), ("vv", "w_vv", "v")]
    for idx, (name, wk, kind) in enumerate(second):
        w = np.asarray(inp[wk], f32)
        for t in range(2):
            tap = w[:, :, 0, t] if kind == "h" else w[:, :, t, 0]
            cw[:, (2 * idx + t) * 128 : (2 * idx + t + 1) * 128] = tap.T

    # correction column vectors: -(1/CS) * colsum of conv lhsT
    ncol1 = np.zeros((32, 2), f32)
    ncol1[:, 0] = -cw1[:, 0:128].sum(axis=1) / (128.0 * 12)
    ncol1[:, 1] = -cw1[:, 128:256].sum(axis=1) / (128.0 * 12)
    ncol2 = np.zeros((128, 8), f32)
    cs2 = {"hh": 128 * 8, "hv": 128 * 9, "vh": 128 * 9, "vv": 128 * 8}
    for idx, (name, _, _) in enumerate(second):
        for t in range(2):
            col = 2 * idx + t
            ncol2[:, col] = -cw[:, col * 128 : (col + 1) * 128].sum(axis=1) / cs2[name]

    gam6 = np.zeros((1, 6 * 128), f32)
    for bi, (name, *_rest) in enumerate(BLOCKS):
        gam6[0, bi * 128 : (bi + 1) * 128] = gammas[name]

    # head weights: W1c = [vw1; aw1] (512, 7424), re-tiled [kslice, c, mt*128]
    W1c = np.concatenate(
        [np.asarray(inp["vw1"], f32), np.asarray(inp["aw1"], f32)], axis=0
    )
    hw = np.empty((NK, 128, 512), f32)
    off = 0
    kidx = 0
    for name, _, _, _, _, ho, wo in BLOCKS:
        S = ho * wo
        Wb = W1c[:, off : off + 128 * S].reshape(512, 128, S)
        off += 128 * S
        for s in range(S):
            hw[kidx] = Wb[:, :, s].T  # [c, 512hidden]
            kidx += 1

    # final layer with dueling algebra folded in
    vw2 = np.asarray(inp["vw2"], f32)
    aw2 = np.asarray(inp["aw2"], f32)
    W2c = np.zeros((4, 512), f32)
    W2c[:, 0:256] = vw2[0][None, :]
    W2c[:, 256:512] = aw2 - aw2.mean(axis=0, keepdims=True)
    W2cT = W2c.T
    fw = np.zeros((128, 16), f32)
    for kt in range(4):
        fw[:, kt * 4 : (kt + 1) * 4] = W2cT[kt * 128 : (kt + 1) * 128, :]
    b2 = (
        np.asarray(inp["vb2"], f32)[0]
        + np.asarray(inp["ab2"], f32)
        - np.asarray(inp["ab2"], f32).mean()
    ).reshape(4, 1)
    hb = np.concatenate(
        [np.asarray(inp["vb1"], f32), np.asarray(inp["ab1"], f32)]
    ).reshape(4, 128).T.copy()

    return {
        "cw1": cw1.astype(BF16),
        "cw": cw.astype(BF16),
        "ncol1": ncol1.astype(BF16),
        "ncol2": ncol2.astype(BF16),
        "gam6": gam6.astype(np.float32),
        "hw": hw.astype(BF16),
        "fw": fw.astype(BF16),
        "hb": hb.astype(np.float32),
        "b2": b2.astype(np.float32),
    }


def _prep_x(xs):
    """Per-core input prep: tap-stacked [c,s,b] bf16 arrays + window sums."""
    f32 = np.float32
    n = xs.shape[0]
    x2 = np.zeros((n, 32, 4, 4), f32)
    x2[:, 0:16] = xs
    x2[:, 16:32, :, 0:3] = xs[:, :, :, 1:4]
    x3 = np.zeros((n, 32, 4, 4), f32)
    x3[:, 0:16] = xs
    x3[:, 16:32, 0:3, :] = xs[:, :, 1:4, :]
    x2 = x2.transpose(1, 2, 3, 0).reshape(32, 16, n)
    x3 = x3.transpose(1, 2, 3, 0).reshape(32, 16, n)
    # window sums over the output grids (h1: j<=2 of 4x4; v1: i<=2)
    x2b = x2.astype(BF16).astype(f32).reshape(32, 4, 4, n)
    x3b = x3.astype(BF16).astype(f32).reshape(32, 4, 4, n)
    pwx = np.zeros((32, 2, n), f32)
    pwx[:, 0] = x2b[:, :, 0:3, :].sum(axis=(1, 2))
    pwx[:, 1] = x3b[:, 0:3, :, :].sum(axis=(1, 2))
    return x2.astype(BF16), x3.astype(BF16), pwx.astype(BF16)


def _prep_x_map(xs, w):
    """Per-core input map: shared weights + this core's prepped x views."""
    x2, x3, pwx = _prep_x(xs)
    m = dict(w)
    m["x2"] = x2
    m["x3"] = x3
    m["pwx"] = pwx
    return m


def _get_nc():
    if "nc" not in _cache:
        _cache["nc"] = _build()
    return _cache["nc"]


def kernel(**inputs) -> np.ndarray:
    from concourse.bass_utils import run_bass_kernel_spmd

    nc = _get_nc()
    x = np.asarray(inputs["x"], np.float32)
    w = _prep_weights(inputs)

    in_maps = []
    for c in range(NCORES):
        xs = x[c * BC : (c + 1) * BC]
        in_maps.append(_prep_x_map(xs, w))

    res = run_bass_kernel_spmd(nc, in_maps, core_ids=list(range(NCORES)))
    out = np.concatenate([r["out"] for r in res.results], axis=0)
    return out.astype(np.float32)
